# revision 24
# baseline (speedup 1.0000x reference)
"""Trainium2 Bass kernel for nn_AuxiliaryYFixed (segment_reduce).

Computes, for z_ins [N,128], sorted bag_idx [N], W [1,128], b [1]:
    loc = z_ins @ W.T + b                      -> [N, 1]
    M[s] = max(loc[i] for bag_idx[i]==s) or 0  -> [B, 1]
returning (M, loc) like the jax reference.

Strategy (8 NeuronCores, data-parallel over N):
  - Each core gets a contiguous shard of 253,952 rows (= 128 partitions x
    L=1984), overlapping 3,952 rows into the next shard (duplicates are
    harmless for max; loc is written only for the owned 250,000 rows).
  - Partition p owns local rows [p*L, (p+1)*L), so loc lands in a [128, L]
    "scan layout".  The projection is split across engines to stay under
    the ~360 GB/s HBM roofline:
      * d < 64 ("low half") on the TensorEngine: pairs of 128-row tiles are
        transpose-packed (stationary = z-pair, moving = identity) into
        zT2 [(j,d) = 128, p = 128]; a second matmul with stationary = zT2
        and moving = W2 [128, 2] emits loc_low directly as [p, j] PSUM
        columns (~1 cycle/row total).
      * d >= 64 ("high half") on DVE: elementwise z*W + grouped reduce.
      * ACT only copies zT2 PSUM->SBUF.
  - Segmented running max via tensor_tensor_scan (op0=add with 0/-1e30
    boundary flags, op1=max), chained per block; cross-partition-row stitch
    with a tiny transposed second scan.
  - Per-segment extraction: group compaction 64:1 (valid because the
    minimum segment length >= 64) computed mostly upfront from idx alone;
    per-group maxima scattered to a [B] partial array with 31 small
    indirect DMAs ([128,1]-index embedding pattern).
  - Host combines: concat loc shards; elementwise max of the 8 partial [B]
    arrays; untouched (empty) bags map to 0.0.
"""

import os
import sys

import numpy as np

for _p in ("/opt/trn_rl_repo", "/root/.axon_site/_ro/trn_rl_repo"):
    if os.path.isdir(_p) and _p not in sys.path:
        sys.path.insert(0, _p)

import concourse.bacc as bacc
import concourse.bass as bass
import concourse.mybir as mybir
import concourse.tile as tile
from concourse.bass import IndirectOffsetOnAxis
from concourse.tile import add_dep_helper

F32 = mybir.dt.float32
I32 = mybir.dt.int32
AX = mybir.AxisListType
OP = mybir.AluOpType
ACTF = mybir.ActivationFunctionType

# Problem geometry (hardcoded per the harness contract).
N = 2_000_000
D = 128
DH = D // 2                  # engine-split point
B = 16384
NCORES = 8
P = 128
SHARD = N // NCORES          # 250,000 owned rows per core
L = 1984                     # free length per partition (31*64)
NC_PAD = P * L               # 253,952 rows processed per core
JB = 64                      # j-columns per block (4 MiB DMA; == group size)
NJ = L // JB                 # 31 blocks
G = 64                       # extraction group size (min segment length >= G)
NG = L // G                  # 31 groups (group g == block g)
NEG = 1.0e30
OWN_FULL_PARTS = SHARD // L           # 126 full partitions owned
OWN_TAIL = SHARD - OWN_FULL_PARTS * L  # 16 leftover elements in partition 126


def build_program(finalize=True):
    nc = bacc.Bacc("TRN2", target_bir_lowering=False, debug=False)

    z_d = nc.dram_tensor("z", [NC_PAD, D], F32, kind="ExternalInput")
    idx_d = nc.dram_tensor("idxf", [NC_PAD], F32, kind="ExternalInput")
    whi_d = nc.dram_tensor("wfull", [P, D], F32, kind="ExternalInput")
    brep_d = nc.dram_tensor("brep", [P, 1], F32, kind="ExternalInput")
    eye_d = nc.dram_tensor("eye", [P, P], F32, kind="ExternalInput")
    nextv_d = nc.dram_tensor("nextv", [1, 1], F32, kind="ExternalInput")

    loc_d = nc.dram_tensor("loc_out", [SHARD], F32, kind="ExternalOutput")
    mc_d = nc.dram_tensor("mc_out", [B], F32, kind="ExternalOutput")

    z_v = z_d[:].rearrange("(p l) d -> p l d", p=P)
    idx_v = idx_d[:].rearrange("(p l) -> p l", p=P)

    with tile.TileContext(nc) as tc:
        with (
            tc.tile_pool(name="zp", bufs=2) as zp,
            tc.tile_pool(name="prodp", bufs=1) as prodp,
            tc.tile_pool(name="cst", bufs=1) as cst,
            tc.tile_pool(name="main", bufs=1) as main,
            tc.tile_pool(name="sm", bufs=1) as sm,
            tc.tile_pool(name="ps2", bufs=1, space="PSUM") as ps2,
        ):
            # ---- constants + idx ----
            whi_t = cst.tile([P, D], F32)
            nc.sync.dma_start(whi_t[:], whi_d[:])
            brep_t = cst.tile([P, 1], F32)
            nc.sync.dma_start(brep_t[:], brep_d[:])
            eye_t = cst.tile([P, P], F32)
            nc.sync.dma_start(eye_t[:], eye_d[:])
            nextv_t = cst.tile([1, 1], F32)
            nc.sync.dma_start(nextv_t[:], nextv_d[:])
            idx_t = main.tile([P, L], F32)
            nc.sync.dma_start(idx_t[:], idx_v)

            loc_t = main.tile([P, L], F32)
            m_t = main.tile([P, L], F32)

            # init the partial-max output early (scatters wait on this)
            neg_t = sm.tile([P, B // P], F32)
            nc.vector.memset(neg_t[:], -NEG)
            init_i = nc.gpsimd.dma_start(
                mc_d[:].rearrange("(p q) -> p q", p=P), neg_t[:]
            )

            # ---- upfront idx-only precompute (overlaps first DMAs) ----
            with nc.named_scope("pre"):
                bnd = main.tile([P, L], F32)
                nc.vector.tensor_tensor(
                    bnd[:, 1:L], idx_t[:, 1:L], idx_t[:, 0:L - 1], op=OP.is_equal
                )
                nc.vector.tensor_scalar(
                    bnd[:, 1:L], bnd[:, 1:L], 1.0, NEG, op0=OP.subtract, op1=OP.mult
                )
                nc.vector.memset(bnd[:, 0:1], -NEG)

                islast = main.tile([P, L], F32)
                nc.vector.tensor_tensor(
                    islast[:, 0:L - 1], idx_t[:, 0:L - 1], idx_t[:, 1:L],
                    op=OP.not_equal,
                )
                nc.vector.memset(islast[:, L - 1:L], 0.0)  # fixed in tail

                # offsets per group: Og = groupmax(islast*(idx+1) - 1)
                wk1 = main.tile([P, L], F32)
                nc.vector.tensor_scalar(wk1[:], idx_t[:], 1.0, None, op0=OP.add)
                nc.vector.tensor_tensor(wk1[:], wk1[:], islast[:], op=OP.mult)
                nc.vector.tensor_scalar(wk1[:], wk1[:], 1.0, None, op0=OP.subtract)
                Og = sm.tile([P, NG], F32)
                nc.vector.tensor_reduce(
                    Og[:], wk1[:].rearrange("p (g k) -> p g k", k=G),
                    axis=AX.X, op=OP.max,
                )
                offs_f = sm.tile([P, NG], F32)
                nc.vector.tensor_scalar(
                    offs_f[:], Og[:], 0.0, 2.0e6, op0=OP.is_lt, op1=OP.mult
                )
                nc.vector.tensor_tensor(offs_f[:], offs_f[:], Og[:], op=OP.add)
                offs_i = sm.tile([P, NG], I32)
                nc.vector.tensor_copy(offs_i[:], offs_f[:])

                # Mg mask: 0 where the group's last-position is in the row's
                # first segment (carry applies), -NEG otherwise
                eqf = main.tile([P, L], F32)
                nc.vector.tensor_scalar(
                    eqf[:], idx_t[:], idx_t[:, 0:1], None, op0=OP.is_equal
                )
                nc.vector.tensor_tensor(wk1[:], islast[:], eqf[:], op=OP.mult)
                nc.vector.tensor_scalar(
                    wk1[:], wk1[:], 1.0, NEG, op0=OP.subtract, op1=OP.mult
                )
                Mg = sm.tile([P, NG], F32)
                nc.vector.tensor_reduce(
                    Mg[:], wk1[:].rearrange("p (g k) -> p g k", k=G),
                    axis=AX.X, op=OP.max,
                )

            # ---- main loop: projection + chained scan, one block per group ----
            with nc.named_scope("proj"):
                for t in range(NJ):
                    j0 = t * JB
                    zt = zp.tile([P, JB * D], F32)
                    nc.sync.dma_start(zt[:], z_v[:, j0:j0 + JB, :])
                    zt3 = zt[:].rearrange("p (j d) -> p j d", d=D)

                    # full-D elementwise mul (W broadcast) + grouped reduce
                    pr = prodp.tile([P, JB * D], F32)
                    nc.vector.tensor_tensor(
                        pr[:].rearrange("p (j d) -> p j d", d=D),
                        zt3,
                        whi_t[:].rearrange("p (o d) -> p o d", o=1)
                            .to_broadcast([P, JB, D]),
                        op=OP.mult,
                    )
                    red = sm.tile([P, JB], F32, tag="red")
                    nc.vector.tensor_reduce(
                        red[:], pr[:].rearrange("p (j d) -> p j d", d=D),
                        axis=AX.X, op=OP.add,
                    )
                    nc.vector.tensor_scalar(
                        loc_t[:, j0:j0 + JB], red[:], brep_t[:, 0:1], None,
                        op0=OP.add,
                    )
                    # chained segmented running max
                    init_state = -NEG if t == 0 else m_t[:, j0 - 1:j0]
                    nc.vector.tensor_tensor_scan(
                        m_t[:, j0:j0 + JB], bnd[:, j0:j0 + JB],
                        loc_t[:, j0:j0 + JB],
                        initial=init_state, op0=OP.add, op1=OP.max,
                    )

            # write owned loc rows out
            nc.sync.dma_start(
                loc_d[0:OWN_FULL_PARTS * L].rearrange(
                    "(p l) -> p l", p=OWN_FULL_PARTS),
                loc_t[0:OWN_FULL_PARTS, :],
            )
            nc.sync.dma_start(
                loc_d[OWN_FULL_PARTS * L:SHARD],
                loc_t[OWN_FULL_PARTS:OWN_FULL_PARTS + 1, 0:OWN_TAIL],
            )

            with nc.named_scope("tail"):
                # ---- stitch across partition rows ----
                pt = ps2.tile([1, 3 * P], F32, tag="pt")
                nc.tensor.matmul(
                    out=pt[:, 0:P], lhsT=m_t[:, L - 1:L], rhs=eye_t[:],
                    start=True, stop=True,
                )
                nc.tensor.matmul(
                    out=pt[:, P:2 * P], lhsT=idx_t[:, 0:1], rhs=eye_t[:],
                    start=True, stop=True,
                )
                nc.tensor.matmul(
                    out=pt[:, 2 * P:3 * P], lhsT=idx_t[:, L - 1:L], rhs=eye_t[:],
                    start=True, stop=True,
                )
                tails = sm.tile([1, P], F32)
                firsts = sm.tile([1, P], F32)
                lasts = sm.tile([1, P], F32)
                nc.vector.tensor_copy(tails[:], pt[:, 0:P])
                nc.vector.tensor_copy(firsts[:], pt[:, P:2 * P])
                nc.vector.tensor_copy(lasts[:], pt[:, 2 * P:3 * P])

                cont = sm.tile([1, P], F32)
                nc.vector.memset(cont[:, 0:1], 0.0)
                nc.vector.tensor_tensor(
                    cont[:, 1:P], firsts[:, 1:P], lasts[:, 0:P - 1], op=OP.is_equal
                )
                single = sm.tile([1, P], F32)
                nc.vector.tensor_tensor(single[:], firsts[:], lasts[:], op=OP.is_equal)
                g_t = sm.tile([1, P], F32)
                nc.vector.tensor_tensor(g_t[:], cont[:], single[:], op=OP.mult)
                nc.vector.tensor_scalar(
                    g_t[:], g_t[:], 1.0, NEG, op0=OP.subtract, op1=OP.mult
                )
                u_t = sm.tile([1, P], F32)
                nc.vector.tensor_tensor_scan(
                    u_t[:], g_t[:], tails[:], initial=-NEG, op0=OP.add, op1=OP.max
                )
                carry_r = sm.tile([1, P], F32)
                nc.vector.memset(carry_r[:, 0:1], -NEG)
                nc.vector.tensor_tensor(
                    carry_r[:, 1:P], u_t[:, 0:P - 1], cont[:, 1:P], op=OP.mult
                )
                tmp = sm.tile([1, P], F32)
                nc.vector.tensor_scalar(
                    tmp[:, 1:P], cont[:, 1:P], 1.0, NEG, op0=OP.subtract, op1=OP.mult
                )
                nc.vector.tensor_tensor(
                    carry_r[:, 1:P], carry_r[:, 1:P], tmp[:, 1:P], op=OP.add
                )
                lastf_r = sm.tile([1, P], F32)
                nc.vector.tensor_tensor(
                    lastf_r[:, 0:P - 1], lasts[:, 0:P - 1], firsts[:, 1:P],
                    op=OP.not_equal,
                )
                nc.vector.tensor_tensor(
                    lastf_r[:, P - 1:P], lasts[:, P - 1:P], nextv_t[0:1, 0:1],
                    op=OP.not_equal,
                )
                pc = ps2.tile([P, 2], F32, tag="pc")
                nc.tensor.matmul(
                    out=pc[:, 0:1], lhsT=carry_r[:], rhs=eye_t[0:1, 0:1],
                    start=True, stop=True,
                )
                nc.tensor.matmul(
                    out=pc[:, 1:2], lhsT=lastf_r[:], rhs=eye_t[0:1, 0:1],
                    start=True, stop=True,
                )
                carry_c = sm.tile([P, 1], F32)
                nc.vector.tensor_copy(carry_c[:], pc[:, 0:1])
                nc.vector.tensor_copy(islast[:, L - 1:L], pc[:, 1:2])

                # redo group NG-1 (last column now known) over its slice
                s0 = (NG - 1) * G
                sl = slice(s0, L)
                wks = sm.tile([P, G], F32)
                nc.vector.tensor_scalar(wks[:], idx_t[:, sl], 1.0, None, op0=OP.add)
                nc.vector.tensor_tensor(wks[:], wks[:], islast[:, sl], op=OP.mult)
                nc.vector.tensor_scalar(wks[:], wks[:], 1.0, None, op0=OP.subtract)
                nc.vector.tensor_reduce(
                    Og[:, NG - 1:NG],
                    wks[:].rearrange("p (o k) -> p o k", o=1),
                    axis=AX.X, op=OP.max,
                )
                nc.vector.tensor_scalar(
                    offs_f[:, NG - 1:NG], Og[:, NG - 1:NG], 0.0, 2.0e6,
                    op0=OP.is_lt, op1=OP.mult,
                )
                nc.vector.tensor_tensor(
                    offs_f[:, NG - 1:NG], offs_f[:, NG - 1:NG], Og[:, NG - 1:NG],
                    op=OP.add,
                )
                nc.vector.tensor_copy(offs_i[:, NG - 1:NG], offs_f[:, NG - 1:NG])
                nc.vector.tensor_tensor(wks[:], islast[:, sl], eqf[:, sl], op=OP.mult)
                nc.vector.tensor_scalar(
                    wks[:], wks[:], 1.0, NEG, op0=OP.subtract, op1=OP.mult
                )
                nc.vector.tensor_reduce(
                    Mg[:, NG - 1:NG],
                    wks[:].rearrange("p (o k) -> p o k", o=1),
                    axis=AX.X, op=OP.max,
                )

                # V = islast*m + (islast-1)*NEG; Vg_m = group max
                V_t = bnd  # reuse
                nc.vector.tensor_scalar(
                    V_t[:], islast[:], 1.0, NEG, op0=OP.subtract, op1=OP.mult
                )
                t2_t = eqf  # reuse
                nc.vector.tensor_tensor(t2_t[:], islast[:], m_t[:], op=OP.mult)
                nc.vector.tensor_tensor(V_t[:], V_t[:], t2_t[:], op=OP.add)
                Vg = sm.tile([P, NG], F32)
                nc.vector.tensor_reduce(
                    Vg[:], V_t[:].rearrange("p (g k) -> p g k", k=G),
                    axis=AX.X, op=OP.max,
                )
                # Vg = max(Vg_m, Mg + carry)
                nc.vector.tensor_scalar(
                    Mg[:], Mg[:], carry_c[:, 0:1], None, op0=OP.add
                )
                nc.vector.tensor_tensor(Vg[:], Vg[:], Mg[:], op=OP.max)

                # scatter: one [128,1]-index indirect DMA per group column
                mc_v = mc_d[:].rearrange("(b one) -> b one", one=1)
                for g in range(NG):
                    scat_i = nc.gpsimd.indirect_dma_start(
                        out=mc_v,
                        out_offset=IndirectOffsetOnAxis(
                            ap=offs_i[:, g:g + 1], axis=0
                        ),
                        in_=Vg[:, g:g + 1],
                        in_offset=None,
                        bounds_check=B - 1,
                        oob_is_err=False,
                    )
                    add_dep_helper(
                        scat_i.ins, init_i.ins, sync=True,
                        reason="scatter waits for mc init completion",
                    )
    nc.compile()
    if finalize:
        nc.finalize()
    return nc


_PROGRAM = None


def _get_program():
    global _PROGRAM
    if _PROGRAM is None:
        _PROGRAM = build_program()
    return _PROGRAM


def make_in_maps(z_ins, bag_idx, W, b):
    z = np.asarray(z_ins, dtype=np.float32)
    idxf = np.asarray(bag_idx).astype(np.float32)
    Wf = np.asarray(W, dtype=np.float32).reshape(1, D)
    bf = np.asarray(b, dtype=np.float32).reshape(-1)

    wfull = np.tile(Wf, (P, 1)).astype(np.float32)           # [128, 128]
    brep = np.full((P, 1), bf[0], dtype=np.float32)
    eye = np.eye(P, dtype=np.float32)

    pad = NC_PAD - SHARD                                      # 3,952
    # pad z rows for the last core: loc = -1e4*||W||^2 + b, far below any
    # real loc, and idx continues the final segment so no fake boundary.
    zpad = np.tile((-1.0e4 * Wf).astype(np.float32), (pad, 1))
    s_last = idxf[-1]

    in_maps = []
    for c in range(NCORES):
        s0 = c * SHARD
        if c < NCORES - 1:
            zc = z[s0:s0 + NC_PAD]
            ic = idxf[s0:s0 + NC_PAD]
            nv = np.array([[idxf[s0 + NC_PAD]]], dtype=np.float32)
        else:
            zc = np.concatenate([z[s0:], zpad], axis=0)
            ic = np.concatenate([idxf[s0:], np.full(pad, s_last, np.float32)])
            nv = np.array([[-1.0]], dtype=np.float32)
        in_maps.append({
            "z": np.ascontiguousarray(zc),
            "idxf": np.ascontiguousarray(ic),
            "wfull": wfull,
            "brep": brep,
            "eye": eye,
            "nextv": nv,
        })
    return in_maps


def combine_outputs(results):
    loc = np.concatenate([np.asarray(r["loc_out"]) for r in results])
    mcs = np.stack([np.asarray(r["mc_out"]) for r in results], axis=0)  # [C,B]
    M = np.max(mcs, axis=0)
    M = np.where(M < -1.0e29, np.float32(0.0), M).astype(np.float32)
    return M[:, None], loc[:, None].astype(np.float32)


def kernel(z_ins, bag_idx, W, b):
    from concourse.bass_utils import run_bass_kernel_spmd

    nc = _get_program()
    in_maps = make_in_maps(z_ins, bag_idx, W, b)
    res = run_bass_kernel_spmd(nc, in_maps, core_ids=list(range(NCORES)))
    return combine_outputs(res.results)


# revision 30
# speedup vs baseline: 1.1119x; 1.1119x over previous
"""Trainium2 Bass kernel for nn_AuxiliaryYFixed (segment_reduce).

Computes, for z_ins [N,128], sorted bag_idx [N], W [1,128], b [1]:
    loc = z_ins @ W.T + b                      -> [N, 1]
    M[s] = max(loc[i] for bag_idx[i]==s) or 0  -> [B, 1]
returning (M, loc) like the jax reference.

Strategy (8 NeuronCores, data-parallel over N):
  - Each core gets a contiguous shard of 253,952 rows (= 128 partitions x
    L=1984), overlapping 3,952 rows into the next shard (duplicates are
    harmless for max; loc is written only for the owned 250,000 rows).
  - Partition p owns local rows [p*L, (p+1)*L), so loc lands in a [128, L]
    "scan layout".  The projection is split across engines to stay under
    the ~360 GB/s HBM roofline:
      * d < 64 ("low half") on the TensorEngine: pairs of 128-row tiles are
        transpose-packed (stationary = z-pair, moving = identity) into
        zT2 [(j,d) = 128, p = 128]; a second matmul with stationary = zT2
        and moving = W2 [128, 2] emits loc_low directly as [p, j] PSUM
        columns (~1 cycle/row total).
      * d >= 64 ("high half") on DVE: elementwise z*W + grouped reduce.
      * ACT only copies zT2 PSUM->SBUF.
  - Segmented running max via tensor_tensor_scan (op0=add with 0/-1e30
    boundary flags, op1=max), chained per block; cross-partition-row stitch
    with a tiny transposed second scan.
  - Per-segment extraction: group compaction 64:1 (valid because the
    minimum segment length >= 64) computed mostly upfront from idx alone;
    per-group maxima scattered to a [B] partial array with 31 small
    indirect DMAs ([128,1]-index embedding pattern).
  - Host combines: concat loc shards; elementwise max of the 8 partial [B]
    arrays; untouched (empty) bags map to 0.0.
"""

import os
import sys

import numpy as np

for _p in ("/opt/trn_rl_repo", "/root/.axon_site/_ro/trn_rl_repo"):
    if os.path.isdir(_p) and _p not in sys.path:
        sys.path.insert(0, _p)

import concourse.bacc as bacc
import concourse.bass as bass
import concourse.mybir as mybir
import concourse.tile as tile
from concourse.bass import IndirectOffsetOnAxis
from concourse.tile import add_dep_helper

F32 = mybir.dt.float32
I32 = mybir.dt.int32
AX = mybir.AxisListType
OP = mybir.AluOpType
ACTF = mybir.ActivationFunctionType

# Problem geometry (hardcoded per the harness contract).
N = 2_000_000
D = 128
DH = D // 2                  # engine-split point
B = 16384
NCORES = 8
P = 128
SHARD = N // NCORES          # 250,000 owned rows per core
L = 1984                     # free length per partition (31*64)
NC_PAD = P * L               # 253,952 rows processed per core
JB = 64                      # j-columns per block (4 MiB DMA; == group size)
NJ = L // JB                 # 31 blocks
G = 64                       # extraction group size (min segment length >= G)
NG = L // G                  # 31 groups (group g == block g)
NEG = 1.0e30
OWN_FULL_PARTS = SHARD // L           # 126 full partitions owned
OWN_TAIL = SHARD - OWN_FULL_PARTS * L  # 16 leftover elements in partition 126


def build_program(finalize=True):
    nc = bacc.Bacc("TRN2", target_bir_lowering=False, debug=False)

    z_d = nc.dram_tensor("z", [NC_PAD, D], F32, kind="ExternalInput")
    idx_d = nc.dram_tensor("idxf", [NC_PAD], F32, kind="ExternalInput")
    whi_d = nc.dram_tensor("wfull", [P, D], F32, kind="ExternalInput")
    brep_d = nc.dram_tensor("brep", [P, 1], F32, kind="ExternalInput")
    eye_d = nc.dram_tensor("eye", [P, P], F32, kind="ExternalInput")
    nextv_d = nc.dram_tensor("nextv", [1, 1], F32, kind="ExternalInput")

    loc_d = nc.dram_tensor("loc_out", [SHARD], F32, kind="ExternalOutput")
    mc_d = nc.dram_tensor("mc_out", [B], F32, kind="ExternalOutput")

    z_v = z_d[:].rearrange("(p l) d -> p l d", p=P)
    idx_v = idx_d[:].rearrange("(p l) -> p l", p=P)

    with tile.TileContext(nc) as tc:
        with (
            tc.tile_pool(name="zp", bufs=2) as zp,
            tc.tile_pool(name="prodp", bufs=1) as prodp,
            tc.tile_pool(name="cst", bufs=1) as cst,
            tc.tile_pool(name="main", bufs=1) as main,
            tc.tile_pool(name="sm", bufs=1) as sm,
            tc.tile_pool(name="ps2", bufs=1, space="PSUM") as ps2,
        ):
            # ---- constants + idx ----
            whi_t = cst.tile([P, D], F32)
            nc.sync.dma_start(whi_t[:], whi_d[:])
            brep_t = cst.tile([P, 1], F32)
            nc.sync.dma_start(brep_t[:], brep_d[:])
            eye_t = cst.tile([P, P], F32)
            nc.sync.dma_start(eye_t[:], eye_d[:])
            nextv_t = cst.tile([1, 1], F32)
            nc.sync.dma_start(nextv_t[:], nextv_d[:])
            idx_t = main.tile([P, L], F32)
            nc.sync.dma_start(idx_t[:], idx_v)

            loc_t = main.tile([P, L], F32)
            m_t = main.tile([P, L], F32)

            # init the partial-max output early (scatters wait on this)
            neg_t = sm.tile([P, B // P], F32)
            nc.vector.memset(neg_t[:], -NEG)
            init_i = nc.gpsimd.dma_start(
                mc_d[:].rearrange("(p q) -> p q", p=P), neg_t[:]
            )

            # ---- upfront idx-only precompute (overlaps first DMAs) ----
            with nc.named_scope("pre"):
                bnd = main.tile([P, L], F32)
                nc.vector.tensor_tensor(
                    bnd[:, 1:L], idx_t[:, 1:L], idx_t[:, 0:L - 1], op=OP.is_equal
                )
                nc.vector.tensor_scalar(
                    bnd[:, 1:L], bnd[:, 1:L], 1.0, NEG, op0=OP.subtract, op1=OP.mult
                )
                nc.vector.memset(bnd[:, 0:1], -NEG)

                islast = main.tile([P, L], F32)
                nc.vector.tensor_tensor(
                    islast[:, 0:L - 1], idx_t[:, 0:L - 1], idx_t[:, 1:L],
                    op=OP.not_equal,
                )
                nc.vector.memset(islast[:, L - 1:L], 0.0)  # fixed in tail

                # offsets per group: Og = groupmax(islast*(idx+1) - 1)
                wk1 = main.tile([P, L], F32)
                nc.vector.tensor_scalar(wk1[:], idx_t[:], 1.0, None, op0=OP.add)
                nc.vector.tensor_tensor(wk1[:], wk1[:], islast[:], op=OP.mult)
                nc.vector.tensor_scalar(wk1[:], wk1[:], 1.0, None, op0=OP.subtract)
                Og = sm.tile([P, NG], F32)
                nc.vector.tensor_reduce(
                    Og[:], wk1[:].rearrange("p (g k) -> p g k", k=G),
                    axis=AX.X, op=OP.max,
                )
                eqf = main.tile([P, L], F32)
                nc.vector.tensor_scalar(
                    eqf[:], idx_t[:], idx_t[:, 0:1], None, op0=OP.is_equal
                )
                offs_f = sm.tile([P, NG], F32)
                nc.vector.tensor_scalar(
                    offs_f[:], Og[:], 0.0, 2.0e6, op0=OP.is_lt, op1=OP.mult
                )
                nc.vector.tensor_tensor(offs_f[:], offs_f[:], Og[:], op=OP.add)
                offs_i = sm.tile([P, NG], I32)
                nc.vector.tensor_copy(offs_i[:], offs_f[:])

            # ---- main loop: projection + chained scan, one block per group ----
            with nc.named_scope("proj"):
                for t in range(NJ):
                    j0 = t * JB
                    zt = zp.tile([P, JB * D], F32)
                    nc.sync.dma_start(zt[:], z_v[:, j0:j0 + JB, :])
                    zt3 = zt[:].rearrange("p (j d) -> p j d", d=D)

                    # full-D elementwise mul (W broadcast) + grouped reduce
                    pr = prodp.tile([P, JB * D], F32)
                    nc.vector.tensor_tensor(
                        pr[:].rearrange("p (j d) -> p j d", d=D),
                        zt3,
                        whi_t[:].rearrange("p (o d) -> p o d", o=1)
                            .to_broadcast([P, JB, D]),
                        op=OP.mult,
                    )
                    red = sm.tile([P, JB], F32, tag="red")
                    nc.vector.tensor_reduce(
                        red[:], pr[:].rearrange("p (j d) -> p j d", d=D),
                        axis=AX.X, op=OP.add,
                    )
                    nc.vector.tensor_scalar(
                        loc_t[:, j0:j0 + JB], red[:], brep_t[:, 0:1], None,
                        op0=OP.add,
                    )
                    # chained segmented running max
                    init_state = -NEG if t == 0 else m_t[:, j0 - 1:j0]
                    nc.vector.tensor_tensor_scan(
                        m_t[:, j0:j0 + JB], bnd[:, j0:j0 + JB],
                        loc_t[:, j0:j0 + JB],
                        initial=init_state, op0=OP.add, op1=OP.max,
                    )
                    if t < NJ - 1:
                        # group max of (islast ? m : -NEG), scattered now
                        # (carry-dependent first segments fixed at the end
                        # with a CCE-max scatter)
                        vsel = prodp.tile([P, JB], F32, tag="vsel")
                        nc.vector.tensor_scalar(
                            vsel[:], islast[:, j0:j0 + JB], 1.0, NEG,
                            op0=OP.subtract, op1=OP.mult,
                        )
                        vse2 = prodp.tile([P, JB], F32, tag="vse2")
                        nc.vector.tensor_tensor(
                            vse2[:], islast[:, j0:j0 + JB], m_t[:, j0:j0 + JB],
                            op=OP.mult,
                        )
                        nc.vector.tensor_tensor(
                            vsel[:], vsel[:], vse2[:], op=OP.add
                        )
                        vgc = prodp.tile([P, 1], F32, tag="vgc")
                        nc.vector.tensor_reduce(
                            vgc[:], vsel[:].rearrange("p (o k) -> p o k", o=1),
                            axis=AX.X, op=OP.max,
                        )
                        sc_i = nc.gpsimd.indirect_dma_start(
                            out=mc_d[:].rearrange("(b one) -> b one", one=1),
                            out_offset=IndirectOffsetOnAxis(
                                ap=offs_i[:, t:t + 1], axis=0
                            ),
                            in_=vgc[:],
                            in_offset=None,
                            bounds_check=B - 1,
                            oob_is_err=False,
                        )
                        add_dep_helper(
                            sc_i.ins, init_i.ins, sync=True,
                            reason="scatter waits for mc init completion",
                        )

            # write owned loc rows out
            nc.sync.dma_start(
                loc_d[0:OWN_FULL_PARTS * L].rearrange(
                    "(p l) -> p l", p=OWN_FULL_PARTS),
                loc_t[0:OWN_FULL_PARTS, :],
            )
            nc.sync.dma_start(
                loc_d[OWN_FULL_PARTS * L:SHARD],
                loc_t[OWN_FULL_PARTS:OWN_FULL_PARTS + 1, 0:OWN_TAIL],
            )

            with nc.named_scope("tail"):
                # ---- stitch across partition rows ----
                pt = ps2.tile([1, 3 * P], F32, tag="pt")
                nc.tensor.matmul(
                    out=pt[:, 0:P], lhsT=m_t[:, L - 1:L], rhs=eye_t[:],
                    start=True, stop=True,
                )
                nc.tensor.matmul(
                    out=pt[:, P:2 * P], lhsT=idx_t[:, 0:1], rhs=eye_t[:],
                    start=True, stop=True,
                )
                nc.tensor.matmul(
                    out=pt[:, 2 * P:3 * P], lhsT=idx_t[:, L - 1:L], rhs=eye_t[:],
                    start=True, stop=True,
                )
                tails = sm.tile([1, P], F32)
                firsts = sm.tile([1, P], F32)
                lasts = sm.tile([1, P], F32)
                nc.vector.tensor_copy(tails[:], pt[:, 0:P])
                nc.vector.tensor_copy(firsts[:], pt[:, P:2 * P])
                nc.vector.tensor_copy(lasts[:], pt[:, 2 * P:3 * P])

                cont = sm.tile([1, P], F32)
                nc.vector.memset(cont[:, 0:1], 0.0)
                nc.vector.tensor_tensor(
                    cont[:, 1:P], firsts[:, 1:P], lasts[:, 0:P - 1], op=OP.is_equal
                )
                single = sm.tile([1, P], F32)
                nc.vector.tensor_tensor(single[:], firsts[:], lasts[:], op=OP.is_equal)
                g_t = sm.tile([1, P], F32)
                nc.vector.tensor_tensor(g_t[:], cont[:], single[:], op=OP.mult)
                nc.vector.tensor_scalar(
                    g_t[:], g_t[:], 1.0, NEG, op0=OP.subtract, op1=OP.mult
                )
                u_t = sm.tile([1, P], F32)
                nc.vector.tensor_tensor_scan(
                    u_t[:], g_t[:], tails[:], initial=-NEG, op0=OP.add, op1=OP.max
                )
                carry_r = sm.tile([1, P], F32)
                nc.vector.memset(carry_r[:, 0:1], -NEG)
                nc.vector.tensor_tensor(
                    carry_r[:, 1:P], u_t[:, 0:P - 1], cont[:, 1:P], op=OP.mult
                )
                tmp = sm.tile([1, P], F32)
                nc.vector.tensor_scalar(
                    tmp[:, 1:P], cont[:, 1:P], 1.0, NEG, op0=OP.subtract, op1=OP.mult
                )
                nc.vector.tensor_tensor(
                    carry_r[:, 1:P], carry_r[:, 1:P], tmp[:, 1:P], op=OP.add
                )
                lastf_r = sm.tile([1, P], F32)
                nc.vector.tensor_tensor(
                    lastf_r[:, 0:P - 1], lasts[:, 0:P - 1], firsts[:, 1:P],
                    op=OP.not_equal,
                )
                nc.vector.tensor_tensor(
                    lastf_r[:, P - 1:P], lasts[:, P - 1:P], nextv_t[0:1, 0:1],
                    op=OP.not_equal,
                )
                # row-wise carry-fix targets: first idx of rows whose
                # first segment closes in-row and continues from the previous
                condr = sm.tile([1, P], F32)
                nc.vector.tensor_scalar(
                    condr[:], single[:], 1.0, -1.0, op0=OP.subtract, op1=OP.mult
                )
                sl_t = sm.tile([1, P], F32)
                nc.vector.tensor_tensor(sl_t[:], single[:], lastf_r[:], op=OP.mult)
                nc.vector.tensor_tensor(condr[:], condr[:], sl_t[:], op=OP.add)
                nc.vector.tensor_tensor(condr[:], condr[:], cont[:], op=OP.mult)
                offsr = sm.tile([1, P], F32)
                nc.vector.tensor_tensor(offsr[:], firsts[:], condr[:], op=OP.mult)
                tmp2 = sm.tile([1, P], F32)
                nc.vector.tensor_scalar(
                    tmp2[:], condr[:], 1.0, -2.0e6, op0=OP.subtract, op1=OP.mult
                )
                nc.vector.tensor_tensor(offsr[:], offsr[:], tmp2[:], op=OP.add)
                pc = ps2.tile([P, 4], F32, tag="pc")
                nc.tensor.matmul(
                    out=pc[:, 0:1], lhsT=carry_r[:], rhs=eye_t[0:1, 0:1],
                    start=True, stop=True,
                )
                nc.tensor.matmul(
                    out=pc[:, 1:2], lhsT=lastf_r[:], rhs=eye_t[0:1, 0:1],
                    start=True, stop=True,
                )
                nc.tensor.matmul(
                    out=pc[:, 2:3], lhsT=offsr[:], rhs=eye_t[0:1, 0:1],
                    start=True, stop=True,
                )
                carry_c = sm.tile([P, 1], F32)
                nc.vector.tensor_copy(carry_c[:], pc[:, 0:1])
                nc.vector.tensor_copy(islast[:, L - 1:L], pc[:, 1:2])
                offsb_i = sm.tile([P, 1], I32)
                nc.vector.tensor_copy(offsb_i[:], pc[:, 2:3])

                # redo group NG-1 (last column now known) over its slice
                s0 = (NG - 1) * G
                sl = slice(s0, L)
                wks = sm.tile([P, G], F32)
                nc.vector.tensor_scalar(wks[:], idx_t[:, sl], 1.0, None, op0=OP.add)
                nc.vector.tensor_tensor(wks[:], wks[:], islast[:, sl], op=OP.mult)
                nc.vector.tensor_scalar(wks[:], wks[:], 1.0, None, op0=OP.subtract)
                nc.vector.tensor_reduce(
                    Og[:, NG - 1:NG],
                    wks[:].rearrange("p (o k) -> p o k", o=1),
                    axis=AX.X, op=OP.max,
                )
                nc.vector.tensor_scalar(
                    offs_f[:, NG - 1:NG], Og[:, NG - 1:NG], 0.0, 2.0e6,
                    op0=OP.is_lt, op1=OP.mult,
                )
                nc.vector.tensor_tensor(
                    offs_f[:, NG - 1:NG], offs_f[:, NG - 1:NG], Og[:, NG - 1:NG],
                    op=OP.add,
                )
                nc.vector.tensor_copy(offs_i[:, NG - 1:NG], offs_f[:, NG - 1:NG])

                # last block's candidates (uses the now-known last column)
                j0 = (NJ - 1) * JB
                vsel = prodp.tile([P, JB], F32, tag="vsel")
                nc.vector.tensor_scalar(
                    vsel[:], islast[:, j0:L], 1.0, NEG,
                    op0=OP.subtract, op1=OP.mult,
                )
                vse2 = prodp.tile([P, JB], F32, tag="vse2")
                nc.vector.tensor_tensor(
                    vse2[:], islast[:, j0:L], m_t[:, j0:L], op=OP.mult
                )
                nc.vector.tensor_tensor(vsel[:], vsel[:], vse2[:], op=OP.add)
                vgc = prodp.tile([P, 1], F32, tag="vgc")
                nc.vector.tensor_reduce(
                    vgc[:], vsel[:].rearrange("p (o k) -> p o k", o=1),
                    axis=AX.X, op=OP.max,
                )
                mc_v = mc_d[:].rearrange("(b one) -> b one", one=1)
                sc_i = nc.gpsimd.indirect_dma_start(
                    out=mc_v,
                    out_offset=IndirectOffsetOnAxis(
                        ap=offs_i[:, NG - 1:NG], axis=0
                    ),
                    in_=vgc[:],
                    in_offset=None,
                    bounds_check=B - 1,
                    oob_is_err=False,
                )
                add_dep_helper(
                    sc_i.ins, init_i.ins, sync=True,
                    reason="scatter waits for mc init completion",
                )
                # within-row max of each row's first segment (masked reduce
                # over loc), then the final value = max(carry, that)
                nc.vector.tensor_scalar(
                    wk1[:], eqf[:], 1.0, NEG, op0=OP.subtract, op1=OP.mult
                )
                fs_sel = wk1  # (eqf-1)*NEG + eqf*loc over full L
                nc.vector.tensor_tensor(
                    eqf[:], eqf[:], loc_t[:], op=OP.mult
                )
                nc.vector.tensor_tensor(fs_sel[:], fs_sel[:], eqf[:], op=OP.add)
                fs_max = sm.tile([P, 1], F32)
                nc.vector.tensor_reduce(
                    fs_max[:], fs_sel[:].rearrange("p (o k) -> p o k", o=1),
                    axis=AX.X, op=OP.max,
                )
                nc.vector.tensor_tensor(fs_max[:], fs_max[:], carry_c[:], op=OP.max)
                scb_i = nc.gpsimd.indirect_dma_start(
                    out=mc_v,
                    out_offset=IndirectOffsetOnAxis(ap=offsb_i[:], axis=0),
                    in_=fs_max[:],
                    in_offset=None,
                    bounds_check=B - 1,
                    oob_is_err=False,
                )
                add_dep_helper(
                    scb_i.ins, init_i.ins, sync=True,
                    reason="carry-fix scatter waits for mc init",
                )
    nc.compile()
    if finalize:
        nc.finalize()
    return nc


_PROGRAM = None


def _get_program():
    global _PROGRAM
    if _PROGRAM is None:
        _PROGRAM = build_program()
    return _PROGRAM


def make_in_maps(z_ins, bag_idx, W, b):
    z = np.asarray(z_ins, dtype=np.float32)
    idxf = np.asarray(bag_idx).astype(np.float32)
    Wf = np.asarray(W, dtype=np.float32).reshape(1, D)
    bf = np.asarray(b, dtype=np.float32).reshape(-1)

    wfull = np.tile(Wf, (P, 1)).astype(np.float32)           # [128, 128]
    brep = np.full((P, 1), bf[0], dtype=np.float32)
    eye = np.eye(P, dtype=np.float32)

    pad = NC_PAD - SHARD                                      # 3,952
    # pad z rows for the last core: loc = -1e4*||W||^2 + b, far below any
    # real loc, and idx continues the final segment so no fake boundary.
    zpad = np.tile((-1.0e4 * Wf).astype(np.float32), (pad, 1))
    s_last = idxf[-1]

    in_maps = []
    for c in range(NCORES):
        s0 = c * SHARD
        if c < NCORES - 1:
            zc = z[s0:s0 + NC_PAD]
            ic = idxf[s0:s0 + NC_PAD]
            nv = np.array([[idxf[s0 + NC_PAD]]], dtype=np.float32)
        else:
            zc = np.concatenate([z[s0:], zpad], axis=0)
            ic = np.concatenate([idxf[s0:], np.full(pad, s_last, np.float32)])
            nv = np.array([[-1.0]], dtype=np.float32)
        in_maps.append({
            "z": np.ascontiguousarray(zc),
            "idxf": np.ascontiguousarray(ic),
            "wfull": wfull,
            "brep": brep,
            "eye": eye,
            "nextv": nv,
        })
    return in_maps


def combine_outputs(results):
    loc = np.concatenate([np.asarray(r["loc_out"]) for r in results])
    mcs = np.stack([np.asarray(r["mc_out"]) for r in results], axis=0)  # [C,B]
    M = np.max(mcs, axis=0)
    M = np.where(M < -1.0e29, np.float32(0.0), M).astype(np.float32)
    return M[:, None], loc[:, None].astype(np.float32)


def kernel(z_ins, bag_idx, W, b):
    from concourse.bass_utils import run_bass_kernel_spmd

    nc = _get_program()
    in_maps = make_in_maps(z_ins, bag_idx, W, b)
    res = run_bass_kernel_spmd(nc, in_maps, core_ids=list(range(NCORES)))
    return combine_outputs(res.results)


# revision 33
# speedup vs baseline: 1.1131x; 1.0011x over previous
"""Trainium2 Bass kernel for nn_AuxiliaryYFixed (segment_reduce).

Computes, for z_ins [N,128], sorted bag_idx [N], W [1,128], b [1]:
    loc = z_ins @ W.T + b                      -> [N, 1]
    M[s] = max(loc[i] for bag_idx[i]==s) or 0  -> [B, 1]
returning (M, loc) like the jax reference.

Strategy (8 NeuronCores, data-parallel over N):
  - Each core gets a contiguous shard of 253,952 rows (= 128 partitions x
    L=1984), overlapping 3,952 rows into the next shard (duplicates are
    harmless for max; loc is written only for the owned 250,000 rows).
  - Partition p owns local rows [p*L, (p+1)*L), so loc lands in a [128, L]
    "scan layout".  The projection is split across engines to stay under
    the ~360 GB/s HBM roofline:
      * d < 64 ("low half") on the TensorEngine: pairs of 128-row tiles are
        transpose-packed (stationary = z-pair, moving = identity) into
        zT2 [(j,d) = 128, p = 128]; a second matmul with stationary = zT2
        and moving = W2 [128, 2] emits loc_low directly as [p, j] PSUM
        columns (~1 cycle/row total).
      * d >= 64 ("high half") on DVE: elementwise z*W + grouped reduce.
      * ACT only copies zT2 PSUM->SBUF.
  - Segmented running max via tensor_tensor_scan (op0=add with 0/-1e30
    boundary flags, op1=max), chained per block; cross-partition-row stitch
    with a tiny transposed second scan.
  - Per-segment extraction: group compaction 64:1 (valid because the
    minimum segment length >= 64) computed mostly upfront from idx alone;
    per-group maxima scattered to a [B] partial array with 31 small
    indirect DMAs ([128,1]-index embedding pattern).
  - Host combines: concat loc shards; elementwise max of the 8 partial [B]
    arrays; untouched (empty) bags map to 0.0.
"""

import os
import sys

import numpy as np

for _p in ("/opt/trn_rl_repo", "/root/.axon_site/_ro/trn_rl_repo"):
    if os.path.isdir(_p) and _p not in sys.path:
        sys.path.insert(0, _p)

import concourse.bacc as bacc
import concourse.bass as bass
import concourse.mybir as mybir
import concourse.tile as tile
from concourse.bass import IndirectOffsetOnAxis
from concourse.tile import add_dep_helper

F32 = mybir.dt.float32
I32 = mybir.dt.int32
AX = mybir.AxisListType
OP = mybir.AluOpType
ACTF = mybir.ActivationFunctionType

# Problem geometry (hardcoded per the harness contract).
N = 2_000_000
D = 128
DH = D // 2                  # engine-split point
B = 16384
NCORES = 8
P = 128
SHARD = N // NCORES          # 250,000 owned rows per core
L = 1984                     # free length per partition (31*64)
NC_PAD = P * L               # 253,952 rows processed per core
JB = 64                      # j-columns per block (4 MiB DMA; == group size)
NJ = L // JB                 # 31 blocks
G = 64                       # extraction group size (min segment length >= G)
NG = L // G                  # 31 groups (group g == block g)
NEG = 1.0e30
OWN_FULL_PARTS = SHARD // L           # 126 full partitions owned
OWN_TAIL = SHARD - OWN_FULL_PARTS * L  # 16 leftover elements in partition 126


def build_program(finalize=True):
    nc = bacc.Bacc("TRN2", target_bir_lowering=False, debug=False)

    z_d = nc.dram_tensor("z", [NC_PAD, D], F32, kind="ExternalInput")
    idx_d = nc.dram_tensor("idxf", [NC_PAD], F32, kind="ExternalInput")
    whi_d = nc.dram_tensor("wfull", [P, D], F32, kind="ExternalInput")
    brep_d = nc.dram_tensor("brep", [P, 1], F32, kind="ExternalInput")
    eye_d = nc.dram_tensor("eye", [P, P], F32, kind="ExternalInput")
    nextv_d = nc.dram_tensor("nextv", [1, 1], F32, kind="ExternalInput")

    loc_d = nc.dram_tensor("loc_out", [SHARD], F32, kind="ExternalOutput")
    mc_d = nc.dram_tensor("mc_out", [B], F32, kind="ExternalOutput")

    z_v = z_d[:].rearrange("(p l) d -> p l d", p=P)
    idx_v = idx_d[:].rearrange("(p l) -> p l", p=P)

    with tile.TileContext(nc) as tc:
        with (
            tc.tile_pool(name="zp", bufs=2) as zp,
            tc.tile_pool(name="prodp", bufs=1) as prodp,
            tc.tile_pool(name="cst", bufs=1) as cst,
            tc.tile_pool(name="main", bufs=1) as main,
            tc.tile_pool(name="sm", bufs=1) as sm,
            tc.tile_pool(name="ps2", bufs=1, space="PSUM") as ps2,
        ):
            # ---- constants + idx ----
            whi_t = cst.tile([P, D], F32)
            nc.sync.dma_start(whi_t[:], whi_d[:])
            brep_t = cst.tile([P, 1], F32)
            nc.sync.dma_start(brep_t[:], brep_d[:])
            eye_t = cst.tile([P, P], F32)
            nc.sync.dma_start(eye_t[:], eye_d[:])
            nextv_t = cst.tile([1, 1], F32)
            nc.sync.dma_start(nextv_t[:], nextv_d[:])
            idx_t = main.tile([P, L], F32)
            nc.sync.dma_start(idx_t[:], idx_v)

            loc_t = main.tile([P, L], F32)
            m_t = main.tile([P, L], F32)

            # init the partial-max output early (scatters wait on this)
            neg_t = sm.tile([P, B // P], F32)
            nc.vector.memset(neg_t[:], -NEG)
            init_i = nc.gpsimd.dma_start(
                mc_d[:].rearrange("(p q) -> p q", p=P), neg_t[:]
            )

            # ---- upfront idx-only precompute (overlaps first DMAs) ----
            with nc.named_scope("pre"):
                bnd = main.tile([P, L], F32)
                nc.vector.tensor_tensor(
                    bnd[:, 1:L], idx_t[:, 1:L], idx_t[:, 0:L - 1], op=OP.is_equal
                )
                nc.vector.tensor_scalar(
                    bnd[:, 1:L], bnd[:, 1:L], 1.0, NEG, op0=OP.subtract, op1=OP.mult
                )
                nc.vector.memset(bnd[:, 0:1], -NEG)

                islast = main.tile([P, L], F32)
                nc.vector.tensor_tensor(
                    islast[:, 0:L - 1], idx_t[:, 0:L - 1], idx_t[:, 1:L],
                    op=OP.not_equal,
                )
                nc.vector.memset(islast[:, L - 1:L], 0.0)  # fixed in tail

                # offsets per group: Og = groupmax(islast*(idx+1) - 1)
                wk1 = main.tile([P, L], F32)
                nc.vector.tensor_scalar(wk1[:], idx_t[:], 1.0, None, op0=OP.add)
                nc.vector.tensor_tensor(wk1[:], wk1[:], islast[:], op=OP.mult)
                nc.vector.tensor_scalar(wk1[:], wk1[:], 1.0, None, op0=OP.subtract)
                Og = sm.tile([P, NG], F32)
                nc.vector.tensor_reduce(
                    Og[:], wk1[:].rearrange("p (g k) -> p g k", k=G),
                    axis=AX.X, op=OP.max,
                )
                eqf = main.tile([P, L], F32)
                nc.vector.tensor_scalar(
                    eqf[:], idx_t[:], idx_t[:, 0:1], None, op0=OP.is_equal
                )
                offs_f = sm.tile([P, NG], F32)
                nc.vector.tensor_scalar(
                    offs_f[:], Og[:], 0.0, 2.0e6, op0=OP.is_lt, op1=OP.mult
                )
                nc.vector.tensor_tensor(offs_f[:], offs_f[:], Og[:], op=OP.add)
                offs_i = sm.tile([P, NG], I32)
                nc.vector.tensor_copy(offs_i[:], offs_f[:])

            # ---- main loop: projection + chained scan, one block per group ----
            with nc.named_scope("proj"):
                for t in range(NJ):
                    j0 = t * JB
                    zt = zp.tile([P, JB * D], F32)
                    nc.sync.dma_start(zt[:], z_v[:, j0:j0 + JB, :])
                    zt3 = zt[:].rearrange("p (j d) -> p j d", d=D)

                    # full-D elementwise mul (W broadcast) + grouped reduce
                    pr = prodp.tile([P, JB * D], F32)
                    nc.vector.tensor_tensor(
                        pr[:].rearrange("p (j d) -> p j d", d=D),
                        zt3,
                        whi_t[:].rearrange("p (o d) -> p o d", o=1)
                            .to_broadcast([P, JB, D]),
                        op=OP.mult,
                    )
                    red = sm.tile([P, JB], F32, tag="red")
                    nc.vector.tensor_reduce(
                        red[:], pr[:].rearrange("p (j d) -> p j d", d=D),
                        axis=AX.X, op=OP.add,
                    )
                    nc.vector.tensor_scalar(
                        loc_t[:, j0:j0 + JB], red[:], brep_t[:, 0:1], None,
                        op0=OP.add,
                    )
                    # chained segmented running max
                    init_state = -NEG if t == 0 else m_t[:, j0 - 1:j0]
                    nc.vector.tensor_tensor_scan(
                        m_t[:, j0:j0 + JB], bnd[:, j0:j0 + JB],
                        loc_t[:, j0:j0 + JB],
                        initial=init_state, op0=OP.add, op1=OP.max,
                    )
                    if t < NJ - 1:
                        # group max of (islast ? m : -NEG), scattered now
                        # (carry-dependent first segments fixed at the end
                        # with a CCE-max scatter)
                        vsel = prodp.tile([P, JB], F32, tag="vsel")
                        nc.vector.tensor_scalar(
                            vsel[:], islast[:, j0:j0 + JB], 1.0, NEG,
                            op0=OP.subtract, op1=OP.mult,
                        )
                        vse2 = prodp.tile([P, JB], F32, tag="vse2")
                        nc.vector.tensor_tensor(
                            vse2[:], islast[:, j0:j0 + JB], m_t[:, j0:j0 + JB],
                            op=OP.mult,
                        )
                        nc.vector.tensor_tensor(
                            vsel[:], vsel[:], vse2[:], op=OP.add
                        )
                        vgc = prodp.tile([P, 1], F32, tag="vgc")
                        nc.vector.tensor_reduce(
                            vgc[:], vsel[:].rearrange("p (o k) -> p o k", o=1),
                            axis=AX.X, op=OP.max,
                        )
                        sc_i = nc.gpsimd.indirect_dma_start(
                            out=mc_d[:].rearrange("(b one) -> b one", one=1),
                            out_offset=IndirectOffsetOnAxis(
                                ap=offs_i[:, t:t + 1], axis=0
                            ),
                            in_=vgc[:],
                            in_offset=None,
                            bounds_check=B - 1,
                            oob_is_err=False,
                        )
                        add_dep_helper(
                            sc_i.ins, init_i.ins, sync=True,
                            reason="scatter waits for mc init completion",
                        )

            # write owned loc rows out
            nc.sync.dma_start(
                loc_d[0:OWN_FULL_PARTS * L].rearrange(
                    "(p l) -> p l", p=OWN_FULL_PARTS),
                loc_t[0:OWN_FULL_PARTS, :],
            )
            nc.sync.dma_start(
                loc_d[OWN_FULL_PARTS * L:SHARD],
                loc_t[OWN_FULL_PARTS:OWN_FULL_PARTS + 1, 0:OWN_TAIL],
            )

            with nc.named_scope("tail"):
                # ---- stitch across partition rows ----
                pt = ps2.tile([1, 3 * P], F32, tag="pt")
                nc.tensor.matmul(
                    out=pt[:, 0:P], lhsT=m_t[:, L - 1:L], rhs=eye_t[:],
                    start=True, stop=True,
                )
                nc.tensor.matmul(
                    out=pt[:, P:2 * P], lhsT=idx_t[:, 0:1], rhs=eye_t[:],
                    start=True, stop=True,
                )
                nc.tensor.matmul(
                    out=pt[:, 2 * P:3 * P], lhsT=idx_t[:, L - 1:L], rhs=eye_t[:],
                    start=True, stop=True,
                )
                tails = sm.tile([1, P], F32)
                firsts = sm.tile([1, P], F32)
                lasts = sm.tile([1, P], F32)
                nc.vector.tensor_copy(tails[:], pt[:, 0:P])
                nc.vector.tensor_copy(firsts[:], pt[:, P:2 * P])
                nc.vector.tensor_copy(lasts[:], pt[:, 2 * P:3 * P])

                cont = sm.tile([1, P], F32)
                nc.vector.memset(cont[:, 0:1], 0.0)
                nc.vector.tensor_tensor(
                    cont[:, 1:P], firsts[:, 1:P], lasts[:, 0:P - 1], op=OP.is_equal
                )
                single = sm.tile([1, P], F32)
                nc.vector.tensor_tensor(single[:], firsts[:], lasts[:], op=OP.is_equal)
                g_t = sm.tile([1, P], F32)
                nc.vector.tensor_tensor(g_t[:], cont[:], single[:], op=OP.mult)
                nc.vector.tensor_scalar(
                    g_t[:], g_t[:], 1.0, NEG, op0=OP.subtract, op1=OP.mult
                )
                u_t = sm.tile([1, P], F32)
                nc.vector.tensor_tensor_scan(
                    u_t[:], g_t[:], tails[:], initial=-NEG, op0=OP.add, op1=OP.max
                )
                carry_r = sm.tile([1, P], F32)
                nc.vector.memset(carry_r[:, 0:1], -NEG)
                nc.vector.tensor_tensor(
                    carry_r[:, 1:P], u_t[:, 0:P - 1], cont[:, 1:P], op=OP.mult
                )
                tmp = sm.tile([1, P], F32)
                nc.vector.tensor_scalar(
                    tmp[:, 1:P], cont[:, 1:P], 1.0, NEG, op0=OP.subtract, op1=OP.mult
                )
                nc.vector.tensor_tensor(
                    carry_r[:, 1:P], carry_r[:, 1:P], tmp[:, 1:P], op=OP.add
                )
                lastf_r = sm.tile([1, P], F32)
                nc.vector.tensor_tensor(
                    lastf_r[:, 0:P - 1], lasts[:, 0:P - 1], firsts[:, 1:P],
                    op=OP.not_equal,
                )
                nc.vector.tensor_tensor(
                    lastf_r[:, P - 1:P], lasts[:, P - 1:P], nextv_t[0:1, 0:1],
                    op=OP.not_equal,
                )
                # row-wise carry-fix targets: first idx of rows whose
                # first segment closes in-row and continues from the previous
                condr = sm.tile([1, P], F32)
                nc.vector.tensor_scalar(
                    condr[:], single[:], 1.0, -1.0, op0=OP.subtract, op1=OP.mult
                )
                sl_t = sm.tile([1, P], F32)
                nc.vector.tensor_tensor(sl_t[:], single[:], lastf_r[:], op=OP.mult)
                nc.vector.tensor_tensor(condr[:], condr[:], sl_t[:], op=OP.add)
                nc.vector.tensor_tensor(condr[:], condr[:], cont[:], op=OP.mult)
                offsr = sm.tile([1, P], F32)
                nc.vector.tensor_tensor(offsr[:], firsts[:], condr[:], op=OP.mult)
                tmp2 = sm.tile([1, P], F32)
                nc.vector.tensor_scalar(
                    tmp2[:], condr[:], 1.0, -2.0e6, op0=OP.subtract, op1=OP.mult
                )
                nc.vector.tensor_tensor(offsr[:], offsr[:], tmp2[:], op=OP.add)
                pc = ps2.tile([P, 4], F32, tag="pc")
                nc.tensor.matmul(
                    out=pc[:, 0:1], lhsT=carry_r[:], rhs=eye_t[0:1, 0:1],
                    start=True, stop=True,
                )
                nc.tensor.matmul(
                    out=pc[:, 1:2], lhsT=lastf_r[:], rhs=eye_t[0:1, 0:1],
                    start=True, stop=True,
                )
                nc.tensor.matmul(
                    out=pc[:, 2:3], lhsT=offsr[:], rhs=eye_t[0:1, 0:1],
                    start=True, stop=True,
                )
                carry_c = sm.tile([P, 1], F32)
                nc.vector.tensor_copy(carry_c[:], pc[:, 0:1])
                nc.vector.tensor_copy(islast[:, L - 1:L], pc[:, 1:2])
                offsb_i = sm.tile([P, 1], I32)
                nc.vector.tensor_copy(offsb_i[:], pc[:, 2:3])

                # redo group NG-1 (last column now known) over its slice
                s0 = (NG - 1) * G
                sl = slice(s0, L)
                wks = sm.tile([P, G], F32)
                nc.vector.tensor_scalar(wks[:], idx_t[:, sl], 1.0, None, op0=OP.add)
                nc.vector.tensor_tensor(wks[:], wks[:], islast[:, sl], op=OP.mult)
                nc.vector.tensor_scalar(wks[:], wks[:], 1.0, None, op0=OP.subtract)
                nc.vector.tensor_reduce(
                    Og[:, NG - 1:NG],
                    wks[:].rearrange("p (o k) -> p o k", o=1),
                    axis=AX.X, op=OP.max,
                )
                nc.vector.tensor_scalar(
                    offs_f[:, NG - 1:NG], Og[:, NG - 1:NG], 0.0, 2.0e6,
                    op0=OP.is_lt, op1=OP.mult,
                )
                nc.vector.tensor_tensor(
                    offs_f[:, NG - 1:NG], offs_f[:, NG - 1:NG], Og[:, NG - 1:NG],
                    op=OP.add,
                )
                nc.vector.tensor_copy(offs_i[:, NG - 1:NG], offs_f[:, NG - 1:NG])

                # last block's candidates (uses the now-known last column)
                j0 = (NJ - 1) * JB
                vsel = prodp.tile([P, JB], F32, tag="vsel")
                nc.vector.tensor_scalar(
                    vsel[:], islast[:, j0:L], 1.0, NEG,
                    op0=OP.subtract, op1=OP.mult,
                )
                vse2 = prodp.tile([P, JB], F32, tag="vse2")
                nc.vector.tensor_tensor(
                    vse2[:], islast[:, j0:L], m_t[:, j0:L], op=OP.mult
                )
                nc.vector.tensor_tensor(vsel[:], vsel[:], vse2[:], op=OP.add)
                vgc = prodp.tile([P, 1], F32, tag="vgc")
                nc.vector.tensor_reduce(
                    vgc[:], vsel[:].rearrange("p (o k) -> p o k", o=1),
                    axis=AX.X, op=OP.max,
                )
                mc_v = mc_d[:].rearrange("(b one) -> b one", one=1)
                sc_i = nc.gpsimd.indirect_dma_start(
                    out=mc_v,
                    out_offset=IndirectOffsetOnAxis(
                        ap=offs_i[:, NG - 1:NG], axis=0
                    ),
                    in_=vgc[:],
                    in_offset=None,
                    bounds_check=B - 1,
                    oob_is_err=False,
                )
                add_dep_helper(
                    sc_i.ins, init_i.ins, sync=True,
                    reason="scatter waits for mc init completion",
                )
                # within-row max of each row's first segment (masked reduce
                # over loc), then the final value = max(carry, that)
                nc.vector.tensor_scalar(
                    wk1[:], eqf[:], 1.0, NEG, op0=OP.subtract, op1=OP.mult
                )
                fs_sel = wk1  # (eqf-1)*NEG + eqf*loc over full L
                nc.vector.tensor_tensor(
                    eqf[:], eqf[:], loc_t[:], op=OP.mult
                )
                nc.vector.tensor_tensor(fs_sel[:], fs_sel[:], eqf[:], op=OP.add)
                fs_max = sm.tile([P, 1], F32)
                nc.vector.tensor_reduce(
                    fs_max[:], fs_sel[:].rearrange("p (o k) -> p o k", o=1),
                    axis=AX.X, op=OP.max,
                )
                nc.vector.tensor_tensor(fs_max[:], fs_max[:], carry_c[:], op=OP.max)
                scb_i = nc.gpsimd.indirect_dma_start(
                    out=mc_v,
                    out_offset=IndirectOffsetOnAxis(ap=offsb_i[:], axis=0),
                    in_=fs_max[:],
                    in_offset=None,
                    bounds_check=B - 1,
                    oob_is_err=False,
                )
                add_dep_helper(
                    scb_i.ins, init_i.ins, sync=True,
                    reason="carry-fix scatter waits for mc init",
                )
    nc.compile()
    if finalize:
        nc.finalize()
    return nc


_PROGRAM = None


def _get_program():
    global _PROGRAM
    if _PROGRAM is None:
        _PROGRAM = build_program()
    return _PROGRAM


def make_in_maps(z_ins, bag_idx, W, b):
    z = np.asarray(z_ins, dtype=np.float32)
    idxf = np.asarray(bag_idx).astype(np.float32)
    Wf = np.asarray(W, dtype=np.float32).reshape(1, D)
    bf = np.asarray(b, dtype=np.float32).reshape(-1)

    wfull = np.tile(Wf, (P, 1)).astype(np.float32)           # [128, 128]
    brep = np.full((P, 1), bf[0], dtype=np.float32)
    eye = np.eye(P, dtype=np.float32)

    pad = NC_PAD - SHARD                                      # 3,952
    # pad z rows for the last core: loc = -1e4*||W||^2 + b, far below any
    # real loc, and idx continues the final segment so no fake boundary.
    zpad = np.tile((-1.0e4 * Wf).astype(np.float32), (pad, 1))
    s_last = idxf[-1]

    in_maps = []
    for c in range(NCORES):
        s0 = c * SHARD
        if c < NCORES - 1:
            zc = z[s0:s0 + NC_PAD]
            ic = idxf[s0:s0 + NC_PAD]
            nv = np.array([[idxf[s0 + NC_PAD]]], dtype=np.float32)
        else:
            zc = np.concatenate([z[s0:], zpad], axis=0)
            ic = np.concatenate([idxf[s0:], np.full(pad, s_last, np.float32)])
            nv = np.array([[-1.0]], dtype=np.float32)
        in_maps.append({
            "z": np.ascontiguousarray(zc),
            "idxf": np.ascontiguousarray(ic),
            "wfull": wfull,
            "brep": brep,
            "eye": eye,
            "nextv": nv,
        })
    return in_maps


def combine_outputs(results):
    loc = np.concatenate([np.asarray(r["loc_out"]) for r in results])
    mcs = np.stack([np.asarray(r["mc_out"]) for r in results], axis=0)  # [C,B]
    M = np.max(mcs, axis=0)
    M = np.where(M < -1.0e29, np.float32(0.0), M).astype(np.float32)
    return M[:, None], loc[:, None].astype(np.float32)


def kernel(z_ins, bag_idx, W, b):
    from concourse.bass_utils import run_bass_kernel_spmd

    nc = _get_program()
    in_maps = make_in_maps(z_ins, bag_idx, W, b)
    res = run_bass_kernel_spmd(nc, in_maps, core_ids=list(range(NCORES)))
    return combine_outputs(res.results)
